# revision 50
# baseline (speedup 1.0000x reference)
"""MoE top-2 routed FFN (B=4, S=2048, D=1024, H=2048, E=8) on 8 TRN2 NeuronCores.

Strategy (expert-parallel, matching the sharding hint):
  - Host computes the tiny gate (softmax top-2) and builds per-expert token
    lists ("all-to-all dispatch" done at the sharding step).
  - Capacity-factor-1.0 dispatch: each core processes exactly CAP=2048
    token-expert pairs (= N*TOP_K/E, perfectly load balanced).  The few
    overflow pairs of overloaded experts (~1% of pairs; the smallest-
    coefficient ones) are computed exactly in fp32 on the host during the
    combine step, so there is no drop error.
  - Core e runs a dense FFN  out = coef * ((relu(x@W1.T)^2 * (x@W3.T)) @ W2.T)
    over its CAP tokens.  Matmuls run in bf16 with fp32 PSUM accumulation;
    coefficients stay fp32, outputs are stored bf16.
  - Host scatter-adds the per-expert outputs back ("combine").

Per-core kernel structure (single pass, weights read once):
  phase 1: for each of 16 H-tiles m: psA = W1m @ xT, psB = W3m @ xT (PSUM),
           gT[m] = relu(psA)^2 * psB  (DVE, bf16)   [H, CAP] layout
  phase 2: for each 128-token tile: out[tok, :] = (gT.T @ W2T) * coef

DMA plan: input APs are pre-packed on the host to match SBUF tile layouts
exactly, so each logical load is ONE dma_start (descriptor generation on the
SP sequencer costs ~600ns per instruction, serially, and transfers execute
in trigger order at ~340 GB/s/core).  Triggers are emitted in PE consumption
order; the head of the kernel is HBM-bandwidth-bound.
"""

import os
import sys

import numpy as np

if os.path.isdir("/opt/trn_rl_repo") and "/opt/trn_rl_repo" not in sys.path:
    sys.path.insert(0, "/opt/trn_rl_repo")

import ml_dtypes

import concourse.bacc as bacc
import concourse.mybir as mybir
from concourse.bass_utils import run_bass_kernel_spmd
from concourse.tile import TileContext

B, S, D, H, E = 4, 2048, 1024, 2048, 8
N = B * S
P = 128
KT = D // P   # 8 contraction tiles over D
MT = H // P   # 16 tiles over H
CAP = N * 2 // E   # 2048: per-core capacity (capacity factor 1.0)
TT = CAP // P      # 16 token tiles
NG = CAP // 512    # 4 moving-dim groups of 512

F32 = mybir.dt.float32
BF16 = mybir.dt.bfloat16
BF16_NP = ml_dtypes.bfloat16

# Set by test harness to capture profiling info.
TRACE = False
LAST_RESULTS = None


def build_kernel():
    nc = bacc.Bacc("TRN2", target_bir_lowering=False)

    # x chunks pre-packed to SBUF layout, one per 512-token group:
    # xg{g}[p, k*512 + c] = x[g*512+c, k*128+p].  Group 0 is split into
    # k-halves (512KB each) so the first psA k-loop can start on the
    # first half while the second is still in flight.
    xg0 = [nc.dram_tensor(f"xg0k{j}", [P, 4 * 512], BF16,
                          kind="ExternalInput") for j in range(2)]
    xg1 = nc.dram_tensor("xg1", [P, KT * 512], BF16, kind="ExternalInput")
    xg2 = nc.dram_tensor("xg2", [P, KT * 512], BF16, kind="ExternalInput")
    xg3 = nc.dram_tensor("xg3", [P, KT * 512], BF16, kind="ExternalInput")
    # w1p[m][d, k*128+h] = W1[m*128+h, k*128+d]; same for W3
    w1p = nc.dram_tensor("w1p", [MT, P, KT * P], BF16, kind="ExternalInput")
    w3p = nc.dram_tensor("w3p", [MT, P, KT * P], BF16, kind="ExternalInput")
    # w2b[b][h, j*1024+d] = W2T[(4b+j)*128+h, d]
    w2b = nc.dram_tensor("w2b", [MT // 4, P, 4 * D], BF16,
                         kind="ExternalInput")
    cf = nc.dram_tensor("cf", [P, TT], F32, kind="ExternalInput")
    out = nc.dram_tensor("out", [TT, 2, P, 512], BF16, kind="ExternalOutput")

    with TileContext(nc) as tc:
        with (
            tc.tile_pool(name="x_pool", bufs=1) as x_pool,
            tc.tile_pool(name="g_pool", bufs=1) as g_pool,
            tc.tile_pool(name="w13_pool", bufs=2) as w13_pool,
            tc.tile_pool(name="w2_pool", bufs=4) as w2_pool,
            tc.tile_pool(name="tmp_pool", bufs=2) as tmp_pool,
            tc.tile_pool(name="ob_pool", bufs=3) as ob_pool,
            tc.tile_pool(name="const_pool", bufs=1) as const_pool,
            tc.tile_pool(name="psAB", bufs=3, space="PSUM") as psAB_pool,
            tc.tile_pool(name="psO", bufs=2, space="PSUM") as psO_pool,
        ):
            # --- PE warmup: flip the HAM clock gate (1.2->2.4GHz) before the
            # first real matmul's data lands (~12us; HBM-bandwidth-bound).
            # NOTE: the HAM boost level is granted from PE duty during the
            # ramp window — 512-col back-to-back accumulation (~100% duty)
            # reaches 2.4GHz; narrow 128-col matmuls (~50% duty from the
            # interleaved LDWEIGHTS) only reach 2.0GHz and the whole run
            # stays there.  12 matmuls bridge until the data arrives. ------
            warm = const_pool.tile([P, 512], BF16, tag="warm")
            nc.any.memset(warm[:], 0.0)
            pswarm = psO_pool.tile([P, 512], F32, tag="psO", name="pswarm")
            NWARM = 12
            for i in range(NWARM):
                nc.tensor.matmul(pswarm[:], warm[:, :P], warm[:],
                                 start=(i == 0), stop=(i == NWARM - 1))
            warmsink = const_pool.tile([P, 1], F32, tag="warmsink")
            nc.vector.tensor_scalar_mul(warmsink[:], pswarm[:, :1], 0.0)

            # --- x loads: 6 triggers, critical pieces first ---------------
            xts = {}   # name -> SBUF tile

            def x_slice(k, g):
                if g == 0:
                    t = xts[f"xg0k{k // 4}"]
                    return t[:, (k % 4) * 512:(k % 4 + 1) * 512]
                return xts[f"xg{g}"][:, k * 512:(k + 1) * 512]

            # --- phase 1: gT[h, tok] = relu(W1 @ xT)^2 * (W3 @ xT) --------
            gts = []
            for m in range(MT):
                gt = g_pool.tile([P, CAP], BF16, tag=f"g{m}", name=f"g_{m}")
                gts.append(gt)

            w2ts = []
            for m in range(MT):
                # The early loads are HBM-bandwidth-bound (~340 GB/s/core,
                # transfers run in trigger order), so triggers are emitted
                # in PE consumption order.
                w1t = w13_pool.tile([P, KT * P], BF16, tag="w1t",
                                    name=f"w1_{m}")
                nc.sync.dma_start(w1t[:], w1p[m])
                if m == 0:
                    for j in range(2):
                        nm = f"xg0k{j}"
                        xt_t = x_pool.tile([P, 4 * 512], BF16, tag=nm,
                                           name=nm)
                        nc.sync.dma_start(xt_t[:], xg0[j][:])
                        xts[nm] = xt_t
                w3t = w13_pool.tile([P, KT * P], BF16, tag="w3t",
                                    name=f"w3_{m}")
                nc.sync.dma_start(w3t[:], w3p[m])
                if m == 0:
                    for nm, dt in (("xg1", xg1), ("xg2", xg2),
                                   ("xg3", xg3)):
                        xt_t = x_pool.tile([P, KT * 512], BF16, tag=nm,
                                           name=nm)
                        nc.sync.dma_start(xt_t[:], dt[:])
                        xts[nm] = xt_t
                # m=0 runs groups singly (x arrival paces the PE anyway);
                # m>=1 pairs groups so consecutive matmuls share the same
                # stationary weight slice -> one LDWEIGHTS per k per pair.
                pairs = ([(g,) for g in range(NG)] if m == 0
                         else [(0, 1), (2, 3)])
                for pr in pairs:
                    psAs = [psAB_pool.tile([P, 512], F32, tag="psA",
                                           name=f"psA_{m}_{g}") for g in pr]
                    psBs = [psAB_pool.tile([P, 512], F32, tag="psB",
                                           name=f"psB_{m}_{g}") for g in pr]
                    for k in range(KT):
                        for g, ps in zip(pr, psAs):
                            nc.tensor.matmul(
                                ps[:],
                                w1t[:, k * P:(k + 1) * P],
                                x_slice(k, g),
                                start=(k == 0),
                                stop=(k == KT - 1),
                            )
                    for k in range(KT):
                        for g, ps in zip(pr, psBs):
                            nc.tensor.matmul(
                                ps[:],
                                w3t[:, k * P:(k + 1) * P],
                                x_slice(k, g),
                                start=(k == 0),
                                stop=(k == KT - 1),
                            )
                    for g, psA, psB in zip(pr, psAs, psBs):
                        r = tmp_pool.tile([P, 512], F32, tag="r",
                                          name=f"r_{m}_{g}")
                        nc.vector.tensor_relu(r[:], psA[:])
                        t2 = tmp_pool.tile([P, 512], F32, tag="t2",
                                           name=f"t2_{m}_{g}")
                        nc.vector.tensor_mul(t2[:], r[:], r[:])
                        nc.vector.tensor_mul(
                            gts[m][:, g * 512:(g + 1) * 512],
                            t2[:],
                            psB[:],
                        )

            # W2 + coef loads emitted after phase-1 DMAs: they ride the idle
            # DMA tail of phase 1, well before phase 2 needs them.
            cft = const_pool.tile([P, TT], F32, tag="cft")
            nc.sync.dma_start(cft[:], cf[:])
            for wb in range(MT // 4):
                w2t = w2_pool.tile([P, 4 * D], BF16, tag="w2t",
                                   name=f"w2_{wb}")
                nc.sync.dma_start(w2t[:], w2b[wb])
                w2ts.append(w2t)

            def w2_slice(hk, c0, cw):
                return w2ts[hk // 4][:, (hk % 4) * D + c0:(hk % 4) * D
                                     + c0 + cw]

            # --- phase 2: out[tok, d] = coef * (g.T @ W2T) ----------------
            for t in range(TT):
                for dg in range(2):
                    if t == TT - 1 and dg == 1:
                        # split the final accumulation into quarters so each
                        # piece's scale+store overlaps the PE's remaining
                        # matmuls, shortening the post-PE tail chain
                        for h in range(4):
                            pso = psO_pool.tile([P, 512], F32, tag="psO",
                                                name=f"psO_{t}_{dg}_{h}")
                            for hk in range(MT):
                                nc.tensor.matmul(
                                    pso[:, :128],
                                    gts[hk][:, t * P:(t + 1) * P],
                                    w2_slice(hk, dg * 512 + h * 128, 128),
                                    start=(hk == 0),
                                    stop=(hk == MT - 1),
                                )
                            ob = ob_pool.tile([P, 512], BF16, tag="ob",
                                              name=f"ob_{t}_{dg}_{h}")
                            nc.vector.tensor_scalar_mul(ob[:, :128],
                                                        pso[:, :128],
                                                        cft[:, t:t + 1])
                            nc.sync.dma_start(
                                out[t, dg][:, h * 128:(h + 1) * 128],
                                ob[:, :128])
                        continue
                    pso = psO_pool.tile([P, 512], F32, tag="psO",
                                        name=f"psO_{t}_{dg}")
                    for hk in range(MT):
                        nc.tensor.matmul(
                            pso[:],
                            gts[hk][:, t * P:(t + 1) * P],
                            w2_slice(hk, dg * 512, 512),
                            start=(hk == 0),
                            stop=(hk == MT - 1),
                        )
                    ob = ob_pool.tile([P, 512], BF16, tag="ob",
                                      name=f"ob_{t}_{dg}")
                    nc.vector.tensor_scalar_mul(ob[:], pso[:],
                                                cft[:, t:t + 1])
                    nc.sync.dma_start(out[t, dg], ob[:])

    if not nc.is_finalized():
        nc.finalize()
    return nc


def kernel(x, W1, W2, W3, gate_w, gate_b):
    global LAST_RESULTS

    xf = np.ascontiguousarray(x.reshape(N, D).astype(np.float32, copy=False))
    W1 = np.asarray(W1, np.float32)
    W2 = np.asarray(W2, np.float32)
    W3 = np.asarray(W3, np.float32)

    # ---- gate: softmax + top-2 (tiny, done on host) ------------------------
    logits = xf @ gate_w.T.astype(np.float32) + gate_b.astype(np.float32)
    logits -= logits.max(axis=-1, keepdims=True)
    probs = np.exp(logits)
    probs /= probs.sum(axis=-1, keepdims=True)
    order = np.argsort(-probs, axis=-1, kind="stable")
    i1, i2 = order[:, 0], order[:, 1]
    ar = np.arange(N)
    p1, p2 = probs[ar, i1], probs[ar, i2]
    ps = p1 + p2
    c1, c2 = p1 / ps, p2 / ps

    # capacity-1.0 dispatch: device takes the CAP largest-coef pairs per
    # expert; the overflow pairs go to the host-exact path.
    idx_list, coef_list, ovf_list = [], [], []
    for e in range(E):
        m1 = i1 == e
        m2 = i2 == e
        ide = np.nonzero(m1 | m2)[0]
        ce = np.where(m1[ide], c1[ide], c2[ide]).astype(np.float32)
        if len(ide) > CAP:
            keep = np.sort(np.argsort(-ce)[:CAP])
            drop = np.setdiff1d(np.arange(len(ide)), keep, assume_unique=True)
            ovf_list.append((e, ide[drop], ce[drop]))
            ide, ce = ide[keep], ce[keep]
        idx_list.append(ide)
        coef_list.append(ce)

    # ---- per-core input packing -------------------------------------------
    in_maps = []
    for e in range(E):
        ide, ce = idx_list[e], coef_list[e]
        ne = len(ide)

        xg = np.zeros((CAP, D), np.float32)
        xg[:ne] = xf[ide]
        r3 = np.ascontiguousarray(xg.T).reshape(KT, P, CAP)

        def xpack(ks, g):
            return np.ascontiguousarray(
                r3[ks, :, g * 512:(g + 1) * 512].transpose(1, 0, 2)
            ).reshape(P, -1).astype(BF16_NP)

        x_np = {f"xg0k{j}": xpack(slice(4 * j, 4 * j + 4), 0)
                for j in range(2)}
        x_np.update({"xg1": xpack(slice(None), 1),
                     "xg2": xpack(slice(None), 2),
                     "xg3": xpack(slice(None), 3)})

        w1e, w3e, w2e = W1[e], W3[e], W2[e]
        # [m, h, k, d] -> [m, d, k, h]: packed[m][d, k*128+h] = W[m*128+h, k*128+d]
        w1p_np = np.ascontiguousarray(
            w1e.reshape(MT, P, KT, P).transpose(0, 3, 2, 1)
        ).reshape(MT, P, KT * P).astype(BF16_NP)
        w3p_np = np.ascontiguousarray(
            w3e.reshape(MT, P, KT, P).transpose(0, 3, 2, 1)
        ).reshape(MT, P, KT * P).astype(BF16_NP)
        # W2T[h, d] tiles packed 4 H-tiles per buffer: [b][h, j*1024+d]
        w2p_np = np.ascontiguousarray(w2e.T).reshape(MT // 4, 4, P, D)
        w2b_np = np.ascontiguousarray(
            w2p_np.transpose(0, 2, 1, 3)).reshape(MT // 4, P, 4 * D).astype(
            BF16_NP)

        cfe = np.zeros(CAP, np.float32)
        cfe[:ne] = ce
        cf_np = np.ascontiguousarray(cfe.reshape(TT, P).T)

        in_maps.append(
            {**x_np, "w1p": w1p_np, "w3p": w3p_np, "w2b": w2b_np,
             "cf": cf_np}
        )

    # ---- build + run on 8 cores -------------------------------------------
    nc = build_kernel()
    res = None
    last_exc = None
    for attempt in range(3):
        try:
            res = run_bass_kernel_spmd(
                nc, in_maps, core_ids=list(range(E)),
                trace=TRACE and attempt == 0,
            )
            break
        except Exception as exc:  # transient device wedge / trace plumbing
            last_exc = exc
    if res is None:
        raise last_exc
    LAST_RESULTS = res

    # ---- combine ----------------------------------------------------------
    out = np.zeros((N, D), np.float32)
    for e in range(E):
        ide = idx_list[e]
        oe = res.results[e]["out"].astype(np.float32)  # [TT, 2, P, 512]
        oe = oe.transpose(0, 2, 1, 3).reshape(CAP, D)
        out[ide] += oe[: len(ide)]

    # host-exact path for capacity-overflow pairs (~1% of pairs)
    for e, ido, co in ovf_list:
        xo = xf[ido]
        ho = np.maximum(xo @ W1[e].T, 0.0) ** 2 * (xo @ W3[e].T)
        out[ido] += co[:, None] * (ho @ W2[e].T)

    return out.reshape(B, S, D)
